# revision 1
# baseline (speedup 1.0000x reference)
"""Trainium2 Bass kernel for nn_DecoderBlock (B=8, S=2048, D=64, H=4, HID=256).

Sharding: data-parallel over batch — each of the 8 NeuronCores processes one
batch element end-to-end (LN1 -> causal MHA -> residual -> LN2 -> FFN ->
residual). No collectives.

Attention is computed in the "transposed score" formulation:
  ST[t, s] = sum_e K[t,e] Q[s,e]  (K^T tile stationary, Q^T streaming)
so softmax probabilities come out as P^T [key t on partitions, query s free],
which is exactly the lhsT/rhs layout the P@V matmul needs — no P transposes.
Softmax skips the max-subtraction (scores are bounded ~|2|, exp is safe) and
gets the denominator for free via a ones-column appended to the V stationary.
Causality: diagonal score tiles are trimmed at 128-granularity in the matmul
and the remaining 128x128 triangle gets -1e9 added via a PE accumulate-matmul
(identity.T @ tri) before the exp.

Walrus in this toolchain only honors ONE sync-wait per instruction; see
_split_multi_waits/_strip_pe_self_waits for the post-scheduling fixups that
make arbitrary Tile programs compile.
"""

import sys

sys.path.insert(0, "/opt/trn_rl_repo")

import numpy as np
from contextlib import ExitStack

import concourse.bass as bass
import concourse.tile as tile
from concourse import mybir

FP = mybir.dt.float32
BF = mybir.dt.bfloat16
AF = mybir.ActivationFunctionType
OP = mybir.AluOpType
AX = mybir.AxisListType

B, S, D, H, HID = 8, 2048, 64, 4, 256
T = S // 128      # 16 token tiles of 128
C = S // 512      # 4 query chunks of 512
PT_BATCH = 4      # key tiles per PT staging buffer
SCALE = 1.0 / np.sqrt(D)
EPS = 1e-5

# Set False to run everything in fp32 (reference-accurate, slower evac).
USE_BF16 = True


def _layernorm(nc, pool, src, dst, g_sb, b_sb, eps_sb):
    """src/dst: SBUF [128, T, 64] fp32. Per-token LN over the last dim."""
    s1 = pool.tile([128, T], FP, tag="ln_s1")
    nc.vector.tensor_reduce(out=s1, in_=src, axis=AX.X, op=OP.add)
    sq = pool.tile([128, T, D], FP, tag="ln_sq")
    nc.vector.tensor_mul(sq, src, src)
    s2 = pool.tile([128, T], FP, tag="ln_s2")
    nc.vector.tensor_reduce(out=s2, in_=sq, axis=AX.X, op=OP.add)
    mu = pool.tile([128, T], FP, tag="ln_mu")
    nc.vector.tensor_scalar_mul(mu, s1, 1.0 / D)
    msq = pool.tile([128, T], FP, tag="ln_msq")
    nc.vector.tensor_scalar_mul(msq, s2, 1.0 / D)
    mu2 = pool.tile([128, T], FP, tag="ln_mu2")
    nc.vector.tensor_mul(mu2, mu, mu)
    var = pool.tile([128, T], FP, tag="ln_var")
    nc.vector.tensor_tensor(out=var, in0=msq, in1=mu2, op=OP.subtract)
    sd = pool.tile([128, T], FP, tag="ln_sd")
    nc.scalar.activation(sd, var, AF.Sqrt, bias=eps_sb)  # sqrt(var + eps)
    rs = pool.tile([128, T], FP, tag="ln_rs")
    nc.vector.reciprocal(rs, sd)
    for i in range(T):
        nc.vector.tensor_scalar(
            out=dst[:, i, :],
            in0=src[:, i, :],
            scalar1=mu[:, i : i + 1],
            scalar2=rs[:, i : i + 1],
            op0=OP.subtract,
            op1=OP.mult,
        )
    if g_sb is not None:
        for i in range(T):
            nc.vector.tensor_mul(dst[:, i, :], dst[:, i, :], g_sb)
    if b_sb is not None:
        for i in range(T):
            nc.vector.tensor_add(dst[:, i, :], dst[:, i, :], b_sb)


def _transpose_to(nc, ctx, tc, pool, src, dst, id_sb, nrow):
    """PE-transpose src [128, T, 64] -> dst [64, S] (rows 0..63).

    nrow rows of dst written; transposes go through PSUM in groups of 4."""
    tp = ctx.enter_context(tc.tile_pool(name="tp_ps", bufs=4, space="PSUM"))
    for g in range(T // 4):
        ps = tp.tile([64, 4, 128], FP, tag="tp")
        for j in range(4):
            # Regular matmul xn_tile.T @ I == transpose; avoids is_transpose
            # codegen, whose LDW struct only fits one sync-wait.
            nc.tensor.matmul(ps[:, j, :], lhsT=src[:, 4 * g + j, :], rhs=id_sb)
        nc.vector.tensor_copy(dst[0:nrow, g * 512 : (g + 1) * 512], ps)



def _split_multi_waits(nc):
    """Hardware TPB instructions have exactly ONE sync-wait slot (the EVENTS
    struct), and this walrus refuses compute instructions carrying more. Hoist
    all-but-one wait of every non-DMA instruction onto same-engine NOPs
    spliced immediately before it (the engine executes the NOPs' waits in
    order, so the dependency semantics are identical)."""
    import bass_rust
    from concourse import mybir as _mb

    eng_api = {
        _mb.EngineType.PE: nc.tensor,
        _mb.EngineType.DVE: nc.vector,
        _mb.EngineType.Activation: nc.scalar,
        _mb.EngineType.Pool: nc.gpsimd,
        _mb.EngineType.SP: nc.sync,
    }
    n_nops = 0
    fn = nc.m.functions[0]
    for bb in fn.blocks:
        out = []
        for ins in list(bb.instructions):
            si = ins.sync_info
            waits = list(si.on_wait) if si and si.on_wait else []
            if len(waits) > 1:
                api = eng_api.get(ins.engine)
                if api is not None:
                    for w in waits[:-1]:
                        nop = api.nop().ins
                        # the engine call appended it to the current bb; pull
                        # it back out and splice it here instead.
                        for b2 in fn.blocks:
                            if b2.instructions and b2.instructions[-1] is nop:
                                b2.instructions.pop()
                                break
                        nop.sync_info = bass_rust.SyncInfo(
                            on_wait=[w], on_update=[]
                        )
                        out.append(nop)
                        n_nops += 1
                    ins.sync_info = bass_rust.SyncInfo(
                        on_wait=[waits[-1]], on_update=list(si.on_update or [])
                    )
            out.append(ins)
        bb.instructions[:] = out
    return n_nops


def _strip_pe_self_waits(nc):
    """Drop S[PE]-waits from PE instructions (Matmult/Ldweights).

    PE never reads PSUM and never writes SBUF, so every PE->PE dependency is a
    PSUM write-after-write, which the in-order pc-monotone PE pipeline already
    orders. Walrus has a single sync-wait slot per matmul (S3_LW struct), so
    these conservative self-waits must go for the kernel to compile."""
    import bass_rust

    stripped = 0
    for f in nc.m.functions:
        for bb in f.blocks:
            for ins in bb.instructions:
                if type(ins).__name__ not in ("InstMatmult", "InstLdweights"):
                    continue
                si = ins.sync_info
                if si is None or not si.on_wait:
                    continue
                kept = [w for w in si.on_wait if not str(w.ant_name).startswith("PE")]
                if len(kept) != len(si.on_wait):
                    ins.sync_info = bass_rust.SyncInfo(
                        on_wait=kept, on_update=list(si.on_update or [])
                    )
                    stripped += 1
    return stripped


def _audit_matmul_waits(nc):
    bad = []
    for f in nc.m.functions:
        for bb in f.blocks:
            for ins in bb.instructions:
                if type(ins).__name__ == "InstMatmult":
                    si = ins.sync_info
                    n = len(si.on_wait) if si and si.on_wait else 0
                    if n > 1:
                        bad.append((ins.name, [str(w.ant_name) for w in si.on_wait]))
    return bad


def build_bass(use_bf16: bool, ln1_identity: bool, ln2_identity: bool) -> bass.Bass:
    dt = BF if use_bf16 else FP
    nc = bass.Bass()

    x_d = nc.declare_dram_parameter("x", [S, D], FP, isOutput=False)
    wq_d = nc.declare_dram_parameter("wq", [D, 256], FP, isOutput=False)
    wk_d = nc.declare_dram_parameter("wk", [D, 256], FP, isOutput=False)
    wv_d = nc.declare_dram_parameter("wv", [D, 256], FP, isOutput=False)
    wo_d = nc.declare_dram_parameter("wo", [256, D], FP, isOutput=False)
    fc1_d = nc.declare_dram_parameter("fc1a", [65, HID], FP, isOutput=False)
    fc2_d = nc.declare_dram_parameter("fc2t", [HID, D], dt, isOutput=False)
    b2_d = nc.declare_dram_parameter("b2rep", [128, 8, D], FP, isOutput=False)
    id_d = nc.declare_dram_parameter("ident", [128, 128], FP, isOutput=False)
    tri_d = nc.declare_dram_parameter("tri", [128, 128], FP, isOutput=False)
    if not ln1_identity:
        g1_d = nc.declare_dram_parameter("g1rep", [128, D], FP, isOutput=False)
        b1_d = nc.declare_dram_parameter("b1rep", [128, D], FP, isOutput=False)
    if not ln2_identity:
        g2_d = nc.declare_dram_parameter("g2rep", [128, D], FP, isOutput=False)
        b2l_d = nc.declare_dram_parameter("b2lrep", [128, D], FP, isOutput=False)
    out_d = nc.declare_dram_parameter("out", [S, D], FP, isOutput=True)

    with tile.TileContext(nc) as tc, ExitStack() as ctx:
        cpool = ctx.enter_context(tc.tile_pool(name="consts", bufs=1))
        apool = ctx.enter_context(tc.tile_pool(name="acts", bufs=1))
        spool = ctx.enter_context(tc.tile_pool(name="small", bufs=1))

        # ---- constants to SBUF
        # Matmul operands are re-homed behind a DVE copy: walrus allows only
        # 2 sync-waits per matmul (1 for transposes), so every matmul operand
        # must present a single producer domain (DVE) instead of DMA queues.
        def _load_dve(name, shape, dtype, src_ap):
            raw = cpool.tile(shape, dtype, name=f"{name}_dma", tag=f"{name}_dma")
            nc.sync.dma_start(raw, src_ap)
            t = cpool.tile(shape, dtype, name=name, tag=name)
            nc.vector.tensor_copy(t, raw)
            return t

        wq_sb = _load_dve("wq", [D, 256], FP, wq_d[:, :])
        wk_sb = _load_dve("wk", [D, 256], FP, wk_d[:, :])
        wv_sb = _load_dve("wv", [D, 256], FP, wv_d[:, :])
        wo_sb = _load_dve("wo", [128, 2, D], FP,
                          wo_d[:, :].rearrange("(g p) e -> p g e", p=128))
        fc1_sb = _load_dve("fc1", [65, HID], FP, fc1_d[:, :])
        fc2_sb = _load_dve("fc2", [128, 2, D], dt,
                           fc2_d[:, :].rearrange("(g p) e -> p g e", p=128))
        b2_sb = cpool.tile([128, 8, D], FP, tag="b2")
        nc.sync.dma_start(b2_sb, b2_d[:, :, :])
        id_sb = _load_dve("ident", [128, 128], FP, id_d[:, :])
        tri_sb = _load_dve("tri", [128, 128], FP, tri_d[:, :])
        g1_sb = b1_sb = g2_sb = b2l_sb = None
        if not ln1_identity:
            g1_sb = cpool.tile([128, D], FP, tag="g1")
            nc.sync.dma_start(g1_sb, g1_d[:, :])
            b1_sb = cpool.tile([128, D], FP, tag="b1")
            nc.sync.dma_start(b1_sb, b1_d[:, :])
        if not ln2_identity:
            g2_sb = cpool.tile([128, D], FP, tag="g2")
            nc.sync.dma_start(g2_sb, g2_d[:, :])
            b2l_sb = cpool.tile([128, D], FP, tag="b2l")
            nc.sync.dma_start(b2l_sb, b2l_d[:, :])

        eps_sb = cpool.tile([128, 1], FP, tag="eps")
        nc.vector.memset(eps_sb, EPS)

        # ---- load x: token-tile-major [128, T, 64]
        x_sb = apool.tile([128, T, D], FP, tag="x")
        nc.sync.dma_start(x_sb, x_d[:, :].rearrange("(i p) d -> p i d", p=128))

        # ---- LN1 -> xn; transpose -> xnT [64, S]
        xn_sb = apool.tile([128, T, D], FP, tag="xn")
        _layernorm(nc, spool, x_sb, xn_sb, g1_sb, b1_sb, eps_sb)
        xnT = apool.tile([D, S], FP, tag="xnT")
        with ExitStack() as c2:
            _transpose_to(nc, c2, tc, spool, xn_sb, xnT, id_sb, D)

        # ---- QKV projections
        # qt/kt pack pr holds heads (2pr, 2pr+1): rows h*64+e, cols tokens.
        qt = [apool.tile([128, S], dt, name=f"qt{p}", tag=f"qt{p}") for p in range(2)]
        kt = [apool.tile([128, S], dt, name=f"kt{p}", tag=f"kt{p}") for p in range(2)]
        # v: token-major with a ones column per head: [128, T, H, 66]
        v_sb = apool.tile([128, T, H, 66], dt, tag="v")
        nc.vector.memset(v_sb[:, :, :, 64:65], 1.0)
        with ExitStack() as c2:
            qk_ps = c2.enter_context(tc.tile_pool(name="qk_ps", bufs=2, space="PSUM"))
            for pr in range(2):
                for w_sb, dst in ((wq_sb, qt[pr]), (wk_sb, kt[pr])):
                    ps = qk_ps.tile([128, 4, 512], FP, tag="qkps")
                    for c4 in range(4):
                        nc.tensor.matmul(
                            ps[:, c4, :],
                            lhsT=w_sb[:, pr * 128 : (pr + 1) * 128],
                            rhs=xnT[:, c4 * 512 : (c4 + 1) * 512],
                        )
                    nc.vector.tensor_copy(dst[:, :].rearrange("p (a n) -> p a n", a=4), ps)
            for vg in range(2):
                ps = qk_ps.tile([128, 8, 256], FP, tag="qkps")
                for j in range(8):
                    ti = vg * 8 + j
                    nc.tensor.matmul(
                        ps[:, j, :],
                        lhsT=xnT[:, ti * 128 : (ti + 1) * 128],
                        rhs=wv_sb,
                    )
                nc.vector.tensor_copy(
                    v_sb[:, vg * 8 : (vg + 1) * 8, :, 0:64],
                    ps.rearrange("p a (h e) -> p a h e", e=64),
                )

        # ---- attention + output projection, overlapped per head-pair:
        # after pair pr's chunks finish, its softmax-denominator gather,
        # normalize, and Wo partial matmuls are emitted immediately so they
        # overlap the other pair's attention. Wo accumulates g=0 then g=1
        # into PSUM tiles that stay live across the whole region.
        scratch = apool.tile([65, H, S], FP, tag="scratch")
        l_all = apool.tile([2, 2, S], FP, tag="l_all")
        rl = apool.tile([2, 2, S], FP, tag="rl")
        rb = apool.tile([128, 2, S], FP, tag="rb")
        st = apool.tile([128, 2, S], FP, tag="st")
        y_sb = apool.tile([128, T, D], FP, tag="y")
        with ExitStack() as c2:
            sc_pool = c2.enter_context(tc.tile_pool(name="sc_ps", bufs=2, space="PSUM"))
            ot_pool = c2.enter_context(tc.tile_pool(name="ot_ps", bufs=2, space="PSUM"))
            pt_pool = c2.enter_context(tc.tile_pool(name="pt_sb", bufs=2))
            dpool = c2.enter_context(tc.tile_pool(name="dram", bufs=1, space="DRAM"))
            rl_d = dpool.tile([2, 2, S], FP, tag="rl_d")
            for pr in range(2):
                for c in range(C):
                    nki = 4 * c + 4
                    ot = [
                        ot_pool.tile([65, 512], FP, name=f"ot{hh}", tag=f"ot{hh}")
                        for hh in range(2)
                    ]
                    for kb in range(nki // PT_BATCH):
                        pt_t = pt_pool.tile([128, PT_BATCH, 2, 512], dt, tag="pt")
                        for kk in range(PT_BATCH):
                            ki = kb * PT_BATCH + kk
                            j = ki - 4 * c
                            off = 128 * j if j >= 0 else 0
                            sc = sc_pool.tile([128, 2, 512], FP, tag="sc")
                            for hh in range(2):
                                lo, hi = hh * 64, hh * 64 + 64
                                nc.tensor.matmul(
                                    sc[:, hh, off:],
                                    lhsT=kt[pr][lo:hi, ki * 128 : (ki + 1) * 128],
                                    rhs=qt[pr][lo:hi, c * 512 + off : (c + 1) * 512],
                                    start=True,
                                    stop=(j < 0),
                                )
                                if j >= 0:
                                    # Causal mask: accumulate the -1e9 upper
                                    # triangle via PE (I.T @ tri == tri).
                                    nc.tensor.matmul(
                                        sc[:, hh, off : off + 128],
                                        lhsT=id_sb,
                                        rhs=tri_sb,
                                        start=False,
                                        stop=True,
                                    )
                            nc.scalar.activation(
                                out=pt_t[:, kk, :, off:],
                                in_=sc[:, :, off:],
                                func=AF.Exp,
                                scale=SCALE,
                            )
                        for hh in range(2):
                            h = 2 * pr + hh
                            for kk in range(PT_BATCH):
                                ki = kb * PT_BATCH + kk
                                j = ki - 4 * c
                                off = 128 * j if j >= 0 else 0
                                nc.tensor.matmul(
                                    ot[hh][:, off:],
                                    lhsT=v_sb[:, ki, h, 0:65],
                                    rhs=pt_t[:, kk, hh, off:],
                                    start=(ki == 0),
                                    stop=(ki == nki - 1),
                                )
                    for hh in range(2):
                        h = 2 * pr + hh
                        nc.vector.tensor_copy(
                            scratch[:, h, c * 512 : (c + 1) * 512], ot[hh]
                        )
                # pair pr finished -> gather l, normalize, emit Wo partials
                g = pr
                for hh in range(2):
                    h = 2 * pr + hh
                    nc.sync.dma_start(l_all[hh : hh + 1, pr, :], scratch[64:65, h, :])
                    nc.sync.dma_start(
                        st[hh * 64 : (hh + 1) * 64, g, :],
                        scratch[0:64, h, :],
                    )
                nc.vector.reciprocal(rl[:, pr, :], l_all[:, pr, :])
                nc.sync.dma_start(rl_d[:, pr, :], rl[:, pr, :])
                for hh in range(2):
                    srcd = rl_d[hh : hh + 1, pr, :]
                    bcast = bass.AP(
                        tensor=srcd.tensor,
                        offset=srcd.offset,
                        ap=[[0, 64]] + [list(srcd.ap[-1])],
                    )
                    nc.sync.dma_start(rb[hh * 64 : (hh + 1) * 64, g, :], bcast)
                nc.vector.tensor_tensor(
                    out=st[:, g, :], in0=st[:, g, :], in1=rb[:, g, :], op=OP.mult
                )
        # ---- output projection (after attention pools close)
        with ExitStack() as c2:
            wo_ps = c2.enter_context(tc.tile_pool(name="wo_ps", bufs=2, space="PSUM"))
            for wg in range(2):
                ps = wo_ps.tile([128, 8, D], FP, tag="wops")
                for j in range(8):
                    tt = wg * 8 + j
                    for g in range(2):
                        nc.tensor.matmul(
                            ps[:, j, :],
                            lhsT=st[:, g, tt * 128 : (tt + 1) * 128],
                            rhs=wo_sb[:, g, :],
                            start=(g == 0),
                            stop=(g == 1),
                        )
                nc.vector.tensor_tensor(
                    out=y_sb[:, wg * 8 : (wg + 1) * 8, :],
                    in0=ps,
                    in1=x_sb[:, wg * 8 : (wg + 1) * 8, :],
                    op=OP.add,
                )

        # ---- LN2 -> yn -> ynT (with ones row 64 for the fc1 bias trick)
        yn_sb = apool.tile([128, T, D], FP, tag="yn")
        _layernorm(nc, spool, y_sb, yn_sb, g2_sb, b2l_sb, eps_sb)
        ynT = apool.tile([65, S], FP, tag="ynT")
        nc.vector.memset(ynT[64:65, :], 1.0)
        with ExitStack() as c2:
            _transpose_to(nc, c2, tc, spool, yn_sb, ynT, id_sb, D)

        # ---- FFN
        h1t = apool.tile([128, 2, S], dt, tag="h1t")
        out_sb = apool.tile([128, T, D], FP, tag="osb")
        with ExitStack() as c2:
            f1_ps = c2.enter_context(tc.tile_pool(name="f1_ps", bufs=2, space="PSUM"))
            for half in range(2):
                ps = f1_ps.tile([128, 4, 512], FP, tag="f1")
                for c4 in range(4):
                    nc.tensor.matmul(
                        ps[:, c4, :],
                        lhsT=fc1_sb[:, half * 128 : (half + 1) * 128],
                        rhs=ynT[:, c4 * 512 : (c4 + 1) * 512],
                    )
                nc.vector.tensor_scalar_max(
                    out=h1t[:, half, :].rearrange("p (a n) -> p a n", a=4),
                    in0=ps,
                    scalar1=0.0,
                )
        with ExitStack() as c2:
            f2_ps = c2.enter_context(tc.tile_pool(name="f2_ps", bufs=2, space="PSUM"))
            for wg in range(2):
                ps = f2_ps.tile([128, 8, D], FP, tag="f2")
                for j in range(8):
                    tt = wg * 8 + j
                    for half in range(2):
                        nc.tensor.matmul(
                            ps[:, j, :],
                            lhsT=h1t[:, half, tt * 128 : (tt + 1) * 128],
                            rhs=fc2_sb[:, half, :],
                            start=(half == 0),
                            stop=(half == 1),
                        )
                sl = slice(wg * 8, (wg + 1) * 8)
                nc.vector.tensor_tensor(
                    out=out_sb[:, sl, :], in0=ps, in1=y_sb[:, sl, :], op=OP.add
                )
                nc.vector.tensor_tensor(
                    out=out_sb[:, sl, :], in0=out_sb[:, sl, :], in1=b2_sb, op=OP.add
                )

        nc.sync.dma_start(out_d[:, :].rearrange("(i p) d -> p i d", p=128), out_sb)

    _strip_pe_self_waits(nc)
    _split_multi_waits(nc)
    return nc


_CACHE = {}
_EXEC_CACHE = {}


def _get_bass(use_bf16, ln1_id, ln2_id):
    key = (use_bf16, ln1_id, ln2_id)
    if key not in _CACHE:
        _CACHE[key] = build_bass(use_bf16, ln1_id, ln2_id)
    return _CACHE[key]


def _get_executor(key, nc):
    """Build (once) a jitted 8-core executor for the Bass program.

    run_bass_via_pjrt re-traces and re-jits on every call (~700 ms); caching
    the jitted shard_map keeps steady-state calls at transfer+execute cost."""
    if key in _EXEC_CACHE:
        return _EXEC_CACHE[key]
    import jax
    from jax.experimental.shard_map import shard_map
    from jax.sharding import Mesh, PartitionSpec
    from concourse import bass2jax, mybir as _mb

    bass2jax.install_neuronx_cc_hook()
    assert nc.dbg_addr is None
    partition_name = (
        nc.partition_id_tensor.name if nc.partition_id_tensor else None
    )

    in_names, out_names, out_avals = [], [], []
    for alloc in nc.m.functions[0].allocations:
        if not isinstance(alloc, _mb.MemoryLocationSet):
            continue
        name = alloc.memorylocations[0].name
        if alloc.kind == "ExternalInput":
            if name != partition_name:
                in_names.append(name)
        elif alloc.kind == "ExternalOutput":
            out_names.append(name)
            out_avals.append(
                jax.core.ShapedArray(
                    tuple(alloc.tensor_shape), _mb.dt.np(alloc.dtype)
                )
            )
    n_params, n_outs = len(in_names), len(out_names)
    all_names = list(in_names) + list(out_names)
    if partition_name is not None:
        all_names.append(partition_name)

    def _body(*args):
        operands = list(args)
        if partition_name is not None:
            operands.append(bass2jax.partition_id_tensor())
        outs = bass2jax._bass_exec_p.bind(
            *operands,
            out_avals=tuple(out_avals),
            in_names=tuple(all_names),
            out_names=tuple(out_names),
            lowering_input_output_aliases=(),
            sim_require_finite=True,
            sim_require_nnan=True,
            nc=nc,
        )
        return tuple(outs)

    devices = jax.devices()[:B]
    mesh = Mesh(np.asarray(devices), ("core",))
    specs = (PartitionSpec("core"),) * (n_params + n_outs)
    sharded = jax.jit(
        shard_map(
            _body,
            mesh=mesh,
            in_specs=specs,
            out_specs=(PartitionSpec("core"),) * n_outs,
            check_rep=False,
        ),
        donate_argnums=tuple(range(n_params, n_params + n_outs)),
        keep_unused=True,
    )

    def execute(in_maps):
        concat_in = [
            np.concatenate([np.asarray(m[name]) for m in in_maps], axis=0)
            for name in in_names
        ]
        concat_zeros = [
            np.zeros((B * a.shape[0], *a.shape[1:]), a.dtype) for a in out_avals
        ]
        out_arrs = sharded(*concat_in, *concat_zeros)
        full = np.asarray(out_arrs[0])
        return full.reshape(B, *out_avals[0].shape)

    _EXEC_CACHE[key] = execute
    return execute


def _host_prep(inputs, use_bf16):
    """Build the per-core (batch-sharded) input maps."""
    f32 = np.float32
    Wq = np.asarray(inputs["Wq"], f32)
    Wk = np.asarray(inputs["Wk"], f32)
    Wv = np.asarray(inputs["Wv"], f32)
    Wo = np.asarray(inputs["Wo"], f32)
    fc1_w = np.asarray(inputs["fc1_w"], f32)
    fc1_b = np.asarray(inputs["fc1_b"], f32)
    fc2_w = np.asarray(inputs["fc2_w"], f32)
    fc2_b = np.asarray(inputs["fc2_b"], f32)
    x = np.ascontiguousarray(np.asarray(inputs["x"], f32))

    wq = np.concatenate([Wq[h].T for h in range(H)], axis=1)  # [64, 256] (d, h*e)
    wk = np.concatenate([Wk[h].T for h in range(H)], axis=1)
    wv = np.concatenate([Wv[h].T for h in range(H)], axis=1)
    wo = np.ascontiguousarray(Wo.T)  # [256, 64] (h*d, e)
    fc1a = np.concatenate([fc1_w.T, fc1_b[None, :]], axis=0)  # [65, 256]
    fc2t = np.ascontiguousarray(fc2_w.T)  # [256, 64]
    if use_bf16:
        import ml_dtypes

        fc2t = fc2t.astype(ml_dtypes.bfloat16)
    b2rep = np.broadcast_to(fc2_b, (128, 8, D)).copy()
    ident = np.eye(128, dtype=f32)
    # tri[p, r] = 0 where r >= p (keep: query col >= key row), else -1e9.
    tri = np.where(np.arange(128)[None, :] >= np.arange(128)[:, None], 0.0, -1e9)
    tri = tri.astype(f32)

    g1 = np.asarray(inputs["ln1_g"], f32)
    b1 = np.asarray(inputs["ln1_b"], f32)
    g2 = np.asarray(inputs["ln2_g"], f32)
    b2 = np.asarray(inputs["ln2_b"], f32)
    ln1_id = bool(np.all(g1 == 1.0) and np.all(b1 == 0.0))
    ln2_id = bool(np.all(g2 == 1.0) and np.all(b2 == 0.0))

    shared = {
        "wq": wq, "wk": wk, "wv": wv, "wo": wo,
        "fc1a": fc1a, "fc2t": fc2t, "b2rep": b2rep, "ident": ident, "tri": tri,
    }
    if not ln1_id:
        shared["g1rep"] = np.broadcast_to(g1, (128, D)).copy()
        shared["b1rep"] = np.broadcast_to(b1, (128, D)).copy()
    if not ln2_id:
        shared["g2rep"] = np.broadcast_to(g2, (128, D)).copy()
        shared["b2lrep"] = np.broadcast_to(b2, (128, D)).copy()
    in_maps = [{**shared, "x": np.ascontiguousarray(x[b])} for b in range(B)]
    return in_maps, ln1_id, ln2_id


def _get_bench(key, nc):
    """Non-donating jitted executor for benchmarking (device-resident I/O)."""
    ck = (key, "bench")
    if ck in _EXEC_CACHE:
        return _EXEC_CACHE[ck]
    import jax
    from jax.experimental.shard_map import shard_map
    from jax.sharding import Mesh, PartitionSpec
    from concourse import bass2jax, mybir as _mb

    bass2jax.install_neuronx_cc_hook()
    partition_name = nc.partition_id_tensor.name if nc.partition_id_tensor else None
    in_names, out_names, out_avals = [], [], []
    for alloc in nc.m.functions[0].allocations:
        if not isinstance(alloc, _mb.MemoryLocationSet):
            continue
        name = alloc.memorylocations[0].name
        if alloc.kind == "ExternalInput":
            if name != partition_name:
                in_names.append(name)
        elif alloc.kind == "ExternalOutput":
            out_names.append(name)
            out_avals.append(
                jax.core.ShapedArray(tuple(alloc.tensor_shape), _mb.dt.np(alloc.dtype))
            )
    n_params, n_outs = len(in_names), len(out_names)
    all_names = list(in_names) + list(out_names)
    if partition_name is not None:
        all_names.append(partition_name)

    def _body(*args):
        operands = list(args)
        if partition_name is not None:
            operands.append(bass2jax.partition_id_tensor())
        outs = bass2jax._bass_exec_p.bind(
            *operands,
            out_avals=tuple(out_avals),
            in_names=tuple(all_names),
            out_names=tuple(out_names),
            lowering_input_output_aliases=(),
            sim_require_finite=True,
            sim_require_nnan=True,
            nc=nc,
        )
        return tuple(outs)

    devices = jax.devices()[:B]
    mesh = Mesh(np.asarray(devices), ("core",))
    sharded = jax.jit(
        shard_map(
            _body,
            mesh=mesh,
            in_specs=(PartitionSpec("core"),) * (n_params + n_outs),
            out_specs=(PartitionSpec("core"),) * n_outs,
            check_rep=False,
        )
    )
    _EXEC_CACHE[ck] = (sharded, in_names, out_avals)
    return _EXEC_CACHE[ck]


def bench(inputs, use_bf16=USE_BF16, iters=(1, 17), reps=4):
    """Measure per-kernel device time by queueing N async executions chained
    through x (out feeds back in), blocking only at the end. Slope between
    iteration counts cancels dispatch/transfer overhead."""
    import time
    import jax

    in_maps, ln1_id, ln2_id = _host_prep(inputs, use_bf16)
    key = (use_bf16, ln1_id, ln2_id)
    nc = _get_bass(use_bf16, ln1_id, ln2_id)
    sharded, in_names, out_avals = _get_bench(key, nc)
    xi = in_names.index("x")
    concat_in = [
        np.concatenate([np.asarray(m[name]) for m in in_maps], axis=0)
        for name in in_names
    ]
    zeros = [np.zeros((B * a.shape[0], *a.shape[1:]), a.dtype) for a in out_avals]
    dev_in = [jax.device_put(a) for a in concat_in + zeros]
    jax.block_until_ready(dev_in)

    def run_n(n):
        cur = list(dev_in)
        t0 = time.perf_counter()
        out = None
        for _ in range(n):
            outs = sharded(*cur)
            out = outs[0]
            cur[xi] = out
        out.block_until_ready()
        return time.perf_counter() - t0

    run_n(2)  # warm
    walls = {}
    for it in iters:
        best = float("inf")
        for _ in range(reps):
            best = min(best, run_n(it))
        walls[it] = best
    i0, i1 = min(iters), max(iters)
    per_iter = (walls[i1] - walls[i0]) / (i1 - i0)
    return per_iter * 1e9, walls


def run(inputs, use_bf16=USE_BF16):
    in_maps, ln1_id, ln2_id = _host_prep(inputs, use_bf16)
    key = (use_bf16, ln1_id, ln2_id)
    nc = _get_bass(use_bf16, ln1_id, ln2_id)
    execute = _get_executor(key, nc)
    out = execute(in_maps)
    return out.astype(np.float32)


def kernel(**inputs) -> np.ndarray:
    return run(inputs)

